# revision 2
# baseline (speedup 1.0000x reference)
"""CenterLoss forward on 8 Trainium2 NeuronCores.

Reference computation (see problem):
    N = 16*256 = 4096 rows, D = 512, C = 10000 classes
    dist[n] = ||x[n] - centers[labels[n]]||^2
    loss = sum_n clamp(dist[n], 1e-12, 1e12) + N*(C-1)*1e-12
(the constant term comes from the reference clamping the masked-out zero
entries of the full N x C distance matrix to 1e-12 before summing).

Sharding: data-parallel over N. Each of the 8 cores gets 512 rows of x and
labels; centers live (replicated) in each core's DRAM but only the 512
needed rows are read via ONE SWDGE dma_gather — 20 MB of centers never
moves. x and centers stream as bf16.

Per-core layout (row r of the shard lives at partition r//4, block r%4):
 - x arrives as one [128, 2048] bf16 DMA (4 KB contiguous per partition,
   128 descriptors — optimal DMA efficiency) on the ACT HWDGE ring;
 - labels arrive pre-marshalled (host) as int16 in the SWDGE index layout:
   idx[i] = label of row 4*(i%128) + i//128, wrapped [16, 32] and
   replicated to [128, 32] (one 64 B descriptor per partition, SP ring);
 - one gpsimd.dma_gather lands centers[label[r]] for all 512 rows into a
   [128, 4, 512] bf16 tile matching the x layout (994 ns fixed + 512
   descriptors, vs 4x that for per-chunk indirect DMAs);
 - DVE: d = x - g (bf16, 2x rate), sq = d*d (bf16), then one segmented
   tensor_reduce [128,4,512] -> [128,4] f32 row sums;
 - out [128, 4] f32 (2 KB) on the SP ring; host clamps and reduces in f64
   (out.reshape(-1)[r] is shard row r by construction).
"""

import numpy as np

N_CORES = 8
ROWS_TOTAL = 4096
ROWS_PER_CORE = ROWS_TOTAL // N_CORES  # 512
P = 128                                # SBUF partitions
RPP = ROWS_PER_CORE // P               # rows per partition = 4
D = 512
C = 10000
CLAMP_MIN = 1e-12
CLAMP_MAX = 1e12

_NC_CACHE = {}


def _build_nc():
    import concourse.bacc as bacc
    import concourse.tile as tile
    from concourse import mybir

    nc = bacc.Bacc("TRN2", target_bir_lowering=False)

    f32 = mybir.dt.float32
    bf16 = mybir.dt.bfloat16
    x_d = nc.dram_tensor("x", [P, RPP * D], bf16, kind="ExternalInput")
    idx_d = nc.dram_tensor("labels", [P, ROWS_PER_CORE // 16], mybir.dt.int16,
                           kind="ExternalInput")
    cen_d = nc.dram_tensor("centers", [C, D], bf16, kind="ExternalInput")
    out_d = nc.dram_tensor("out", [P, RPP], f32, kind="ExternalOutput")

    with tile.TileContext(nc) as tc:
        with tc.tile_pool(name="io", bufs=1) as io:
            idx_t = io.tile([P, ROWS_PER_CORE // 16], mybir.dt.int16)
            x_t = io.tile([P, RPP * D], bf16)
            g_t = io.tile([P, RPP, D], bf16)
            d_t = io.tile([P, RPP * D], bf16)
            sq_t = io.tile([P, RPP * D], bf16)
            rs_t = io.tile([P, RPP], f32)

            # index block first (the gather's descriptor gen waits on it);
            # 64 B per partition, single DMA on the SP ring.
            nc.sync.dma_start(out=idx_t[:], in_=idx_d[:, :])
            # x as one DMA: 128 x 4 KB contiguous descriptors on the ACT ring.
            nc.scalar.dma_start(out=x_t[:], in_=x_d[:, :])

            # one SWDGE gather for all 512 rows:
            # g_t[p, c, :] = centers[idx[c*128 + p]] = centers[label[4p+c]]
            nc.gpsimd.dma_gather(
                out_ap=g_t[:, :, :],
                in_ap=cen_d[:, :],
                idxs_ap=idx_t[:, :],
                num_idxs=ROWS_PER_CORE,
                num_idxs_reg=ROWS_PER_CORE,
                elem_size=D,
            )

            g_flat = g_t[:, :, :].rearrange("p a b -> p (a b)")
            nc.vector.tensor_sub(d_t[:], x_t[:], g_flat)
            nc.vector.tensor_mul(sq_t[:], d_t[:], d_t[:])
            sq_3d = sq_t[:].rearrange("p (a b) -> p a b", a=RPP, b=D)
            nc.vector.tensor_reduce(
                out=rs_t[:],
                in_=sq_3d,
                axis=mybir.AxisListType.X,
                op=mybir.AluOpType.add,
            )

            nc.sync.dma_start(out=out_d[:, :], in_=rs_t[:])

    nc.finalize()
    return nc


def _get_nc():
    if "nc" not in _NC_CACHE:
        _NC_CACHE["nc"] = _build_nc()
    return _NC_CACHE["nc"]


def _make_in_maps(x, labels, centers):
    import ml_dtypes
    bf16 = ml_dtypes.bfloat16
    xf = np.ascontiguousarray(np.asarray(x).reshape(ROWS_TOTAL, D)
                              .astype(bf16))
    lab = np.asarray(labels).reshape(ROWS_TOTAL).astype(np.int16)
    cen = np.ascontiguousarray(np.asarray(centers).astype(bf16))

    in_maps = []
    for k in range(N_CORES):
        sl = slice(k * ROWS_PER_CORE, (k + 1) * ROWS_PER_CORE)
        xs = xf[sl].reshape(P, RPP * D)
        # idx[i] = label of shard row 4*(i%128) + i//128, then wrapped so
        # wrapped[p, s] = idx[s*16 + p], replicated across the 8 gpsimd
        # core blocks -> [128, 32].
        idx = lab[sl].reshape(P, RPP).T.reshape(ROWS_PER_CORE)
        wrapped = idx.reshape(ROWS_PER_CORE // 16, 16).T
        rep = np.ascontiguousarray(np.tile(wrapped, (8, 1)))
        in_maps.append({"x": xs, "labels": rep, "centers": cen})
    return in_maps


def _collect(results):
    """Device outputs -> full loss (host clamp + reduce)."""
    # out[p, c] = squared distance of shard row 4p + c -> reshape(-1)
    # restores shard row order; cores are concatenated in row order.
    per_row = np.concatenate(
        [r["out"].reshape(-1) for r in results]).astype(np.float64)
    total = np.clip(per_row, CLAMP_MIN, CLAMP_MAX).sum()
    total += ROWS_TOTAL * (C - 1) * CLAMP_MIN
    return np.asarray(total, dtype=np.float32)


def kernel(x, labels, centers):
    import time
    from concourse.bass_utils import run_bass_kernel_spmd

    nc = _get_nc()
    in_maps = _make_in_maps(x, labels, centers)
    last_err = None
    for attempt in range(3):
        if attempt:
            time.sleep(30)  # transient device errors recover in <1 min
        try:
            res = run_bass_kernel_spmd(nc, in_maps,
                                       core_ids=list(range(N_CORES)))
            return _collect(res.results)
        except Exception as e:  # noqa: BLE001 - retry any runtime failure
            last_err = e
    raise last_err


# revision 3
# speedup vs baseline: 1.7896x; 1.7896x over previous
"""CenterLoss forward on 8 Trainium2 NeuronCores.

Reference computation (see problem):
    N = 16*256 = 4096 rows, D = 512, C = 10000 classes
    dist[n] = ||x[n] - centers[labels[n]]||^2
    loss = sum_n clamp(dist[n], 1e-12, 1e12) + N*(C-1)*1e-12
(the constant term comes from the reference clamping the masked-out zero
entries of the full N x C distance matrix to 1e-12 before summing).

Sharding: data-parallel over N. Each of the 8 cores gets 512 rows of x and
labels; centers live (replicated) in each core's DRAM but only the 512
needed rows are read via indirect (gather) DMAs — 20 MB of centers never
moves. x and centers stream as bf16.

Per-core layout: shard row r lives at partition r//4, chunk c = r%4
(so every DMA is one instruction with large contiguous descriptors):
 - x arrives as ONE [128, 4*512] bf16 DMA (4 KB contiguous per partition,
   128 descriptors) on the ACT HWDGE ring;
 - labels arrive as ONE [128, 4] int32 DMA (16 B per partition) on the SP
   ring — column c is exactly the [128,1] offset AP chunk c's gather needs
   (the only offset-AP shape the DGE gathers correctly);
 - 4 indirect gathers (gpsimd SWDGE, default ucode lib — dma_gather lives
   in an overlay lib whose mid-kernel load costs ~12 us) land
   centers[label[4p+c]] into g[:, c, :], pipelining with compute;
 - DVE per chunk: subtract (bf16 2x rate) + fused square-and-row-reduce
   (scalar_tensor_tensor, bf16 product tile, f32 accum_out);
 - out [128, 4] f32 (2 KB) on the SP ring; host clamps and reduces in f64
   (out.reshape(-1)[r] is shard row r by construction).
"""

import numpy as np

N_CORES = 8
ROWS_TOTAL = 4096
ROWS_PER_CORE = ROWS_TOTAL // N_CORES  # 512
P = 128                                # SBUF partitions
RPP = ROWS_PER_CORE // P               # rows per partition = chunks = 4
D = 512
C = 10000
CLAMP_MIN = 1e-12
CLAMP_MAX = 1e12

_NC_CACHE = {}


def _build_nc():
    import concourse.bacc as bacc
    import concourse.bass as bass
    import concourse.tile as tile
    from concourse import mybir

    nc = bacc.Bacc("TRN2", target_bir_lowering=False)

    f32 = mybir.dt.float32
    bf16 = mybir.dt.bfloat16
    x_d = nc.dram_tensor("x", [P, RPP * D], bf16, kind="ExternalInput")
    lab_d = nc.dram_tensor("labels", [P, RPP], mybir.dt.int32,
                           kind="ExternalInput")
    cen_d = nc.dram_tensor("centers", [C, D], bf16, kind="ExternalInput")
    out_d = nc.dram_tensor("out", [P, RPP], f32, kind="ExternalOutput")

    with tile.TileContext(nc) as tc:
        with tc.tile_pool(name="io", bufs=1) as io, \
             tc.tile_pool(name="work", bufs=2) as work:
            lab_t = io.tile([P, RPP], mybir.dt.int32)
            x_t = io.tile([P, RPP, D], bf16)
            g_t = io.tile([P, RPP, D], bf16)
            rs_t = io.tile([P, RPP], f32)

            # labels first (the gathers' descriptor gen waits on them).
            nc.sync.dma_start(out=lab_t[:], in_=lab_d[:, :])
            # x as one DMA: 128 x 4 KB contiguous descriptors.
            nc.scalar.dma_start(out=x_t[:, :, :], in_=x_d[:, :])

            for c in range(RPP):
                nc.gpsimd.indirect_dma_start(
                    out=g_t[:, c, :],
                    out_offset=None,
                    in_=cen_d[:, :],
                    in_offset=bass.IndirectOffsetOnAxis(
                        ap=lab_t[:, c:c + 1], axis=0),
                )

            for c in range(RPP):
                d_t = work.tile([P, D], bf16, tag="d")
                nc.vector.tensor_sub(d_t[:], x_t[:, c, :], g_t[:, c, :])
                sq_t = work.tile([P, D], bf16, tag="sq")
                # sq = (d + 0) * d, accum_out = per-row sum (f32); fused on
                # DVE (tensor_tensor_reduce hits an unsupported ISA opcode
                # on this runtime). bf16 product tile keeps the 2x DVE rate.
                nc.vector.scalar_tensor_tensor(
                    out=sq_t[:],
                    in0=d_t[:],
                    scalar=0.0,
                    in1=d_t[:],
                    op0=mybir.AluOpType.add,
                    op1=mybir.AluOpType.mult,
                    accum_out=rs_t[:, c:c + 1],
                )

            nc.sync.dma_start(out=out_d[:, :], in_=rs_t[:])

    nc.finalize()
    return nc


def _get_nc():
    if "nc" not in _NC_CACHE:
        _NC_CACHE["nc"] = _build_nc()
    return _NC_CACHE["nc"]


def _make_in_maps(x, labels, centers):
    import ml_dtypes
    bf16 = ml_dtypes.bfloat16
    xf = np.ascontiguousarray(np.asarray(x).reshape(ROWS_TOTAL, D)
                              .astype(bf16))
    lab = np.ascontiguousarray(
        np.asarray(labels).reshape(ROWS_TOTAL).astype(np.int32))
    cen = np.ascontiguousarray(np.asarray(centers).astype(bf16))

    in_maps = []
    for k in range(N_CORES):
        sl = slice(k * ROWS_PER_CORE, (k + 1) * ROWS_PER_CORE)
        # row r of the shard -> partition r//4, chunk r%4: plain reshape.
        xs = xf[sl].reshape(P, RPP * D)
        ls = lab[sl].reshape(P, RPP)
        in_maps.append({"x": xs, "labels": ls, "centers": cen})
    return in_maps


def _collect(results):
    """Device outputs -> full loss (host clamp + reduce)."""
    # out[p, c] = squared distance of shard row 4p + c -> reshape(-1)
    # restores shard row order; cores are concatenated in row order.
    per_row = np.concatenate(
        [r["out"].reshape(-1) for r in results]).astype(np.float64)
    total = np.clip(per_row, CLAMP_MIN, CLAMP_MAX).sum()
    total += ROWS_TOTAL * (C - 1) * CLAMP_MIN
    return np.asarray(total, dtype=np.float32)


def kernel(x, labels, centers):
    import time
    from concourse.bass_utils import run_bass_kernel_spmd

    nc = _get_nc()
    in_maps = _make_in_maps(x, labels, centers)
    last_err = None
    for attempt in range(3):
        if attempt:
            time.sleep(30)  # transient device errors recover in <1 min
        try:
            res = run_bass_kernel_spmd(nc, in_maps,
                                       core_ids=list(range(N_CORES)))
            return _collect(res.results)
        except Exception as e:  # noqa: BLE001 - retry any runtime failure
            last_err = e
    raise last_err


# revision 6
# speedup vs baseline: 1.9177x; 1.0716x over previous
"""CenterLoss forward on 8 Trainium2 NeuronCores.

Reference computation (see problem):
    N = 16*256 = 4096 rows, D = 512, C = 10000 classes
    dist[n] = ||x[n] - centers[labels[n]]||^2
    loss = sum_n clamp(dist[n], 1e-12, 1e12) + N*(C-1)*1e-12
(the constant term comes from the reference clamping the masked-out zero
entries of the full N x C distance matrix to 1e-12 before summing).

Sharding: data-parallel over N. Each of the 8 cores gets 512 rows of x and
labels; centers live (replicated) in each core's DRAM but only the 512
needed rows are read via indirect (gather) DMAs — 20 MB of centers never
moves. x and centers stream as bf16.

Raw bass (no TileContext): the kernel is 12 instructions with a linear
dependency chain, and Tile's exit sequence (drain + all-engine barrier +
sem clear + second barrier) costs ~7 us of measured tail. Hand-rolled
semaphores end the program ~300 ns after the output DMA lands instead.

Per-core layout: shard row r lives at partition r//4, chunk c = r%4
(so every DMA is one instruction with large contiguous descriptors):
 - x arrives as ONE [128, 4*512] bf16 DMA (4 KB contiguous per partition,
   128 descriptors) on the ACT HWDGE ring;
 - labels arrive as ONE [128, 4] int32 DMA (16 B per partition) on the SP
   ring — column c is exactly the [128,1] offset AP chunk c's gather needs
   (the only offset-AP shape the DGE gathers correctly);
 - 4 indirect gathers (gpsimd SWDGE, default ucode lib — dma_gather lives
   in an overlay lib whose mid-kernel load costs ~12 us) land
   centers[label[4p+c]] into g[:, c, :], pipelining with compute;
 - DVE per chunk: subtract (bf16 2x rate) + fused square-and-row-reduce
   (scalar_tensor_tensor, bf16 product tile, f32 accum_out);
 - out [128, 4] f32 (2 KB) on the SP ring; host clamps and reduces in f64
   (out.reshape(-1)[r] is shard row r by construction);
 - gpsimd waits for the out DMA then clears the kernel semaphores (the
   repeat-execution contract Tile's exit normally provides).
"""

import numpy as np

N_CORES = 8
ROWS_TOTAL = 4096
ROWS_PER_CORE = ROWS_TOTAL // N_CORES  # 512
P = 128                                # SBUF partitions
RPP = ROWS_PER_CORE // P               # rows per partition = chunks = 4
D = 512
C = 10000
CLAMP_MIN = 1e-12
CLAMP_MAX = 1e12

_NC_CACHE = {}


def _build_nc():
    from contextlib import ExitStack

    import concourse.bacc as bacc
    import concourse.bass as bass
    from concourse import mybir

    nc = bacc.Bacc("TRN2", target_bir_lowering=False)

    f32 = mybir.dt.float32
    bf16 = mybir.dt.bfloat16
    x_d = nc.dram_tensor("x", [P, RPP * D], bf16, kind="ExternalInput")
    lab_d = nc.dram_tensor("labels", [P, RPP], mybir.dt.int32,
                           kind="ExternalInput")
    cen_d = nc.dram_tensor("centers", [C, D], bf16, kind="ExternalInput")
    out_d = nc.dram_tensor("out", [P, RPP], f32, kind="ExternalOutput")

    with ExitStack() as st:
        lab_t = st.enter_context(nc.sbuf_tensor("lab", [P, RPP],
                                                mybir.dt.int32))
        x_t = st.enter_context(nc.sbuf_tensor("xt", [P, RPP, D], bf16))
        g_t = st.enter_context(nc.sbuf_tensor("gt", [P, RPP, D], bf16))
        rs_t = st.enter_context(nc.sbuf_tensor("rs", [P, RPP], f32))
        d_ts = [st.enter_context(nc.sbuf_tensor(f"d{i}", [P, D], bf16))
                for i in range(2)]
        sq_ts = [st.enter_context(nc.sbuf_tensor(f"sq{i}", [P, D], bf16))
                 for i in range(2)]

        s_lab = nc.alloc_semaphore("s_lab")
        s_x = nc.alloc_semaphore("s_x")
        s_g = [nc.alloc_semaphore(f"s_g{c}") for c in range(RPP)]
        s_v = nc.alloc_semaphore("s_v")
        s_out = nc.alloc_semaphore("s_out")

        # labels first (the gathers' descriptor gen waits on them).
        nc.sync.dma_start(out=lab_t[:, :], in_=lab_d[:, :]).then_inc(s_lab, 16)
        # x as one DMA: 128 x 4 KB contiguous descriptors.
        nc.scalar.dma_start(out=x_t[:, :, :], in_=x_d[:, :]).then_inc(s_x, 16)

        nc.gpsimd.wait_ge(s_lab, 16)
        for c in range(RPP):
            nc.gpsimd.indirect_dma_start(
                out=g_t[:, c, :],
                out_offset=None,
                in_=cen_d[:, :],
                in_offset=bass.IndirectOffsetOnAxis(
                    ap=lab_t[:, c:c + 1], axis=0),
            ).then_inc(s_g[c], 16)

        nc.vector.wait_ge(s_x, 16)
        for c in range(RPP):
            nc.vector.wait_ge(s_g[c], 16)
            d_t, sq_t = d_ts[c % 2], sq_ts[c % 2]
            nc.vector.tensor_sub(d_t[:, :], x_t[:, c, :], g_t[:, c, :])
            # sq = (d + 0) * d, accum_out = per-row sum (f32); fused on DVE
            # (tensor_tensor_reduce hits an unsupported ISA opcode on this
            # runtime). bf16 product tile keeps the 2x DVE rate.
            nc.vector.scalar_tensor_tensor(
                out=sq_t[:, :],
                in0=d_t[:, :],
                scalar=0.0,
                in1=d_t[:, :],
                op0=mybir.AluOpType.add,
                op1=mybir.AluOpType.mult,
                accum_out=rs_t[:, c:c + 1],
            ).then_inc(s_v, 1)

        nc.sync.wait_ge(s_v, RPP)
        nc.sync.dma_start(out=out_d[:, :], in_=rs_t[:, :]).then_inc(s_out, 16)

        # Repeat-execution contract: every sem this run bumped must read 0
        # at the next launch. All updates are retired once the out DMA's sem
        # fires (it is causally last), so gpsimd can clear without a barrier.
        nc.gpsimd.wait_ge(s_out, 16)
        nc.clear_and_free_semaphores([s_lab, s_x, *s_g, s_v, s_out])

    nc.finalize()
    return nc


def _get_nc():
    if "nc" not in _NC_CACHE:
        _NC_CACHE["nc"] = _build_nc()
    return _NC_CACHE["nc"]


def _make_in_maps(x, labels, centers):
    import ml_dtypes
    bf16 = ml_dtypes.bfloat16
    xf = np.ascontiguousarray(np.asarray(x).reshape(ROWS_TOTAL, D)
                              .astype(bf16))
    lab = np.ascontiguousarray(
        np.asarray(labels).reshape(ROWS_TOTAL).astype(np.int32))
    cen = np.ascontiguousarray(np.asarray(centers).astype(bf16))

    in_maps = []
    for k in range(N_CORES):
        sl = slice(k * ROWS_PER_CORE, (k + 1) * ROWS_PER_CORE)
        # row r of the shard -> partition r//4, chunk r%4: plain reshape.
        xs = xf[sl].reshape(P, RPP * D)
        ls = lab[sl].reshape(P, RPP)
        in_maps.append({"x": xs, "labels": ls, "centers": cen})
    return in_maps


def _collect(results):
    """Device outputs -> full loss (host clamp + reduce)."""
    # out[p, c] = squared distance of shard row 4p + c -> reshape(-1)
    # restores shard row order; cores are concatenated in row order.
    per_row = np.concatenate(
        [r["out"].reshape(-1) for r in results]).astype(np.float64)
    total = np.clip(per_row, CLAMP_MIN, CLAMP_MAX).sum()
    total += ROWS_TOTAL * (C - 1) * CLAMP_MIN
    return np.asarray(total, dtype=np.float32)


def kernel(x, labels, centers):
    import time
    from concourse.bass_utils import run_bass_kernel_spmd

    nc = _get_nc()
    in_maps = _make_in_maps(x, labels, centers)
    last_err = None
    for attempt in range(3):
        if attempt:
            time.sleep(30)  # transient device errors recover in <1 min
        try:
            res = run_bass_kernel_spmd(nc, in_maps,
                                       core_ids=list(range(N_CORES)))
            return _collect(res.results)
        except Exception as e:  # noqa: BLE001 - retry any runtime failure
            last_err = e
    raise last_err
